# revision 2
# baseline (speedup 1.0000x reference)
"""BottomPool (cumulative max along H) Trainium2 Bass kernel.

Full input x: (16, 256, 128, 128) fp32. out[b,c,h,w] = max_{h'<=h} x[b,c,h',w].

Strategy: data-parallel over the 4096 (b,c) planes -> 512 planes per core.
Per core, planes are mapped [partition p in 0..127] x [q in 0..3] with
plane = q*128 + p. SBUF tiles hold 8 consecutive h-rows for all 512 planes
([128, 4, 8, 128] fp32 = 2MB DMAs). The cummax is a serial chain of
[128, 4*128] DVE tensor_max ops (one per h-row), carried across tiles.
No transposes, no cross-core communication.
"""

import numpy as np

import concourse.tile as tile
from concourse import bacc, mybir
from concourse.bass_utils import run_bass_kernel_spmd

N_CORES = 8
B, C, H, W = 16, 256, 128, 128
P = 128  # SBUF partitions
PLANES_PER_CORE = (B * C) // N_CORES  # 512
HS = 8  # h-rows per SBUF tile / DMA


def build_module(planes=PLANES_PER_CORE, h=H, w=W, hs=HS, n_cores=N_CORES,
                 bufs=3, split=2):
    """Build + compile the per-core Bass module (same program on all cores).

    Loads issue on nc.sync (SP HWDGE ring), stores on nc.scalar (ACT ring)
    so a store blocked on compute never head-of-line-blocks the next load.
    Each hs-row tile's DMA is split into `split` pieces so the first max op
    starts after 1/split of the load and stores drain before the tile ends.
    """
    assert planes % P == 0 and h % hs == 0 and hs % split == 0
    q = planes // P
    ns = h // hs
    hpc = hs // split  # h-rows per DMA chunk
    nc = bacc.Bacc(
        "TRN2", target_bir_lowering=False, debug=False, num_devices=n_cores
    )
    x = nc.dram_tensor(
        "x", [planes, h, w], mybir.dt.float32, kind="ExternalInput"
    ).ap()
    y = nc.dram_tensor(
        "y", [planes, h, w], mybir.dt.float32, kind="ExternalOutput"
    ).ap()
    xv = x.rearrange("(q p) h w -> p q h w", p=P)
    yv = y.rearrange("(q p) h w -> p q h w", p=P)

    with tile.TileContext(nc) as tc:
        with (
            tc.tile_pool(name="pin", bufs=bufs) as pin,
            tc.tile_pool(name="pout", bufs=bufs) as pout,
        ):
            prev = None
            for s in range(ns):
                tin = pin.tile([P, q, hs, w], mybir.dt.float32)
                for c in range(split):
                    lo, hi = c * hpc, (c + 1) * hpc
                    nc.sync.dma_start(
                        tin[:, :, lo:hi, :],
                        xv[:, :, s * hs + lo:s * hs + hi, :],
                    )
                tout = pout.tile([P, q, hs, w], mybir.dt.float32)
                for hh in range(hs):
                    cur = tin[:, :, hh, :]
                    o = tout[:, :, hh, :]
                    if prev is None:
                        nc.vector.tensor_copy(o, cur)
                    else:
                        nc.vector.tensor_max(o, cur, prev)
                    prev = tout[:, :, hh, :]
                    if (hh + 1) % hpc == 0:
                        lo, hi = hh + 1 - hpc, hh + 1
                        nc.scalar.dma_start(
                            yv[:, :, s * hs + lo:s * hs + hi, :],
                            tout[:, :, lo:hi, :],
                        )
    nc.compile()
    return nc


_NC_CACHE = {}


def _get_module():
    if "nc" not in _NC_CACHE:
        _NC_CACHE["nc"] = build_module()
    return _NC_CACHE["nc"]


def kernel(x: np.ndarray) -> np.ndarray:
    assert x.shape == (B, C, H, W), x.shape
    x = np.ascontiguousarray(np.asarray(x), dtype=np.float32)
    flat = x.reshape(B * C, H, W)
    in_maps = [
        {"x": flat[k * PLANES_PER_CORE:(k + 1) * PLANES_PER_CORE]}
        for k in range(N_CORES)
    ]
    nc = _get_module()
    res = run_bass_kernel_spmd(nc, in_maps, list(range(N_CORES)))
    out = np.concatenate([r["y"] for r in res.results], axis=0)
    return out.reshape(B, C, H, W)


# revision 3
# speedup vs baseline: 1.2435x; 1.2435x over previous
"""BottomPool (cumulative max along H) Trainium2 Bass kernel.

Full input x: (16, 256, 128, 128) fp32. out[b,c,h,w] = max_{h'<=h} x[b,c,h',w].

Strategy: data-parallel over the 4096 (b,c) planes -> 512 planes per core.
Per core, planes are mapped [partition p in 0..127] x [q in 0..3] with
plane = q*128 + p. SBUF tiles hold 8 consecutive h-rows for all 512 planes
([128, 4, 8, 128] fp32 = 2MB DMAs). The cummax is a serial chain of
[128, 4*128] DVE tensor_max ops (one per h-row), carried across tiles.
No transposes, no cross-core communication.
"""

import numpy as np

import concourse.tile as tile
from concourse import bacc, mybir
from concourse.bass_utils import run_bass_kernel_spmd

N_CORES = 8
B, C, H, W = 16, 256, 128, 128
P = 128  # SBUF partitions
PLANES_PER_CORE = (B * C) // N_CORES  # 512
HS = 8  # h-rows per SBUF tile / DMA


def build_module(planes=PLANES_PER_CORE, h=H, w=W, hs=16, qt=4,
                 n_cores=N_CORES, bufs_in=3, bufs_out=2, split=1,
                 store_engine="scalar"):
    """Build + compile the per-core Bass module (same program on all cores).

    Layout: plane = q*128 + p; tiles are [128, qt, hs, w] (qt of the
    planes//128 q-groups, hs h-rows). The DMA descriptor contiguous chunk
    is (hs/split)*w*4 bytes — keep it >= 8KB. DVE does one [128, qt*w]
    tensor_max per h-row, serially chained within a q-group.
    Loads issue on nc.sync (SP HWDGE ring); stores on nc.scalar (ACT ring)
    so a store blocked on compute doesn't head-of-line-block loads.
    """
    q = planes // P
    assert planes % P == 0 and h % hs == 0 and hs % split == 0 and q % qt == 0
    nq = q // qt
    ns = h // hs
    hpc = hs // split
    nc = bacc.Bacc(
        "TRN2", target_bir_lowering=False, debug=False, num_devices=n_cores
    )
    x = nc.dram_tensor(
        "x", [planes, h, w], mybir.dt.float32, kind="ExternalInput"
    ).ap()
    y = nc.dram_tensor(
        "y", [planes, h, w], mybir.dt.float32, kind="ExternalOutput"
    ).ap()
    xv = x.rearrange("(q p) h w -> p q h w", p=P)
    yv = y.rearrange("(q p) h w -> p q h w", p=P)

    with tile.TileContext(nc) as tc:
        store_eng = getattr(nc, store_engine)
        with (
            tc.tile_pool(name="pin", bufs=bufs_in) as pin,
            tc.tile_pool(name="pout", bufs=bufs_out) as pout,
        ):
            for qg in range(nq):
                qlo, qhi = qg * qt, (qg + 1) * qt
                prev = None
                for s in range(ns):
                    tin = pin.tile([P, qt, hs, w], mybir.dt.float32)
                    for c in range(split):
                        lo, hi = c * hpc, (c + 1) * hpc
                        nc.sync.dma_start(
                            tin[:, :, lo:hi, :],
                            xv[:, qlo:qhi, s * hs + lo:s * hs + hi, :],
                        )
                    tout = pout.tile([P, qt, hs, w], mybir.dt.float32)
                    for hh in range(hs):
                        cur = tin[:, :, hh, :]
                        o = tout[:, :, hh, :]
                        if prev is None:
                            nc.vector.tensor_copy(o, cur)
                        else:
                            nc.vector.tensor_max(o, cur, prev)
                        prev = tout[:, :, hh, :]
                        if (hh + 1) % hpc == 0:
                            lo, hi = hh + 1 - hpc, hh + 1
                            store_eng.dma_start(
                                yv[:, qlo:qhi, s * hs + lo:s * hs + hi, :],
                                tout[:, :, lo:hi, :],
                            )
    nc.compile()
    return nc


_NC_CACHE = {}


def _get_module():
    if "nc" not in _NC_CACHE:
        _NC_CACHE["nc"] = build_module()
    return _NC_CACHE["nc"]


def kernel(x: np.ndarray) -> np.ndarray:
    assert x.shape == (B, C, H, W), x.shape
    x = np.ascontiguousarray(np.asarray(x), dtype=np.float32)
    flat = x.reshape(B * C, H, W)
    in_maps = [
        {"x": flat[k * PLANES_PER_CORE:(k + 1) * PLANES_PER_CORE]}
        for k in range(N_CORES)
    ]
    nc = _get_module()
    res = run_bass_kernel_spmd(nc, in_maps, list(range(N_CORES)))
    out = np.concatenate([r["y"] for r in res.results], axis=0)
    return out.reshape(B, C, H, W)


# revision 7
# speedup vs baseline: 1.4338x; 1.1530x over previous
"""BottomPool (cumulative max along H) Trainium2 Bass kernel.

Full input x: (16, 256, 128, 128) fp32. out[b,c,h,w] = max_{h'<=h} x[b,c,h',w].

Strategy: data-parallel over the 4096 (b,c) planes -> 512 planes per core.
Per core, planes are mapped [partition p in 0..127] x [q in 0..3] with
plane = q*128 + p. SBUF tiles hold 8 consecutive h-rows for all 512 planes
([128, 4, 8, 128] fp32 = 2MB DMAs). The cummax is a serial chain of
[128, 4*128] DVE tensor_max ops (one per h-row), carried across tiles.
No transposes, no cross-core communication.
"""

import numpy as np

import concourse.tile as tile
from concourse import bacc, mybir
from concourse.bass_utils import run_bass_kernel_spmd

N_CORES = 8
B, C, H, W = 16, 256, 128, 128
P = 128  # SBUF partitions
PLANES_PER_CORE = (B * C) // N_CORES  # 512
HS = 8  # h-rows per SBUF tile / DMA


def build_module(planes=PLANES_PER_CORE, h=H, w=W, hs=16, qt=4,
                 n_cores=N_CORES, bufs_in=3, bufs_out=2,
                 store_engine="scalar", hsegs=None):
    """Build + compile the per-core Bass module (same program on all cores).

    Layout: plane = q*128 + p; tiles are [128, qt, seg, w] (qt of the
    planes//128 q-groups, seg h-rows). The DMA descriptor contiguous chunk
    is seg*w*4 bytes — keep it >= 8KB for the bulk tiles. DVE does one
    [128, qt*w] tensor_max per h-row, serially chained within a q-group.
    Loads issue on nc.sync (SP HWDGE ring); stores on nc.scalar (ACT ring)
    so a store blocked on compute doesn't head-of-line-block loads.
    `hsegs` tapers tile heights at both edges: small first tiles let the
    DVE chain start sooner; small last tiles let the final stores drain
    overlapped with the chain's tail instead of strictly after it.
    """
    q = planes // P
    assert planes % P == 0 and q % qt == 0
    nq = q // qt
    if hsegs is None:
        if h % hs == 0 and h // hs >= 4:
            mid = (h - 32) // hs
            assert (h - 32) % hs == 0
            hsegs = [8, 8] + [hs] * mid + [8, 4, 4]
        else:
            assert h % hs == 0
            hsegs = [hs] * (h // hs)
    assert sum(hsegs) == h, (hsegs, h)
    nc = bacc.Bacc(
        "TRN2", target_bir_lowering=False, debug=False, num_devices=n_cores
    )
    x = nc.dram_tensor(
        "x", [planes, h, w], mybir.dt.float32, kind="ExternalInput"
    ).ap()
    y = nc.dram_tensor(
        "y", [planes, h, w], mybir.dt.float32, kind="ExternalOutput"
    ).ap()
    xv = x.rearrange("(q p) h w -> p q h w", p=P)
    yv = y.rearrange("(q p) h w -> p q h w", p=P)

    with tile.TileContext(nc) as tc:
        store_eng = getattr(nc, store_engine)
        with (
            tc.tile_pool(name="pin", bufs=bufs_in) as pin,
            tc.tile_pool(name="pout", bufs=bufs_out) as pout,
        ):
            for qg in range(nq):
                qlo, qhi = qg * qt, (qg + 1) * qt
                prev = None
                h0 = 0
                for seg in hsegs:
                    # fixed padded_shape so all tiles share one slot size
                    tin = pin.tile([P, qt, seg, w], mybir.dt.float32)
                    nc.sync.dma_start(
                        tin[:], xv[:, qlo:qhi, h0:h0 + seg, :]
                    )
                    tout = pout.tile([P, qt, seg, w], mybir.dt.float32)
                    for hh in range(seg):
                        cur = tin[:, :, hh, :]
                        o = tout[:, :, hh, :]
                        if prev is None:
                            nc.vector.tensor_copy(o, cur)
                        else:
                            nc.vector.tensor_max(o, cur, prev)
                        prev = tout[:, :, hh, :]
                    store_eng.dma_start(
                        yv[:, qlo:qhi, h0:h0 + seg, :], tout[:]
                    )
                    h0 += seg
    nc.compile()
    return nc


_NC_CACHE = {}


def _get_module():
    if "nc" not in _NC_CACHE:
        _NC_CACHE["nc"] = build_module()
    return _NC_CACHE["nc"]


def kernel(x: np.ndarray) -> np.ndarray:
    assert x.shape == (B, C, H, W), x.shape
    x = np.ascontiguousarray(np.asarray(x), dtype=np.float32)
    flat = x.reshape(B * C, H, W)
    in_maps = [
        {"x": flat[k * PLANES_PER_CORE:(k + 1) * PLANES_PER_CORE]}
        for k in range(N_CORES)
    ]
    nc = _get_module()
    res = run_bass_kernel_spmd(nc, in_maps, list(range(N_CORES)))
    out = np.concatenate([r["y"] for r in res.results], axis=0)
    return out.reshape(B, C, H, W)
